# revision 18
# baseline (speedup 1.0000x reference)
"""Trainium2 Bass kernel for nn_DavidBeansV2 (sparse wormhole attention).

Math (per batch item b, derived from the reference):
  xp = x[b, 1:, :]                                  # [P, D]
  q  = l2norm(xp @ Wq + bq); k = l2norm(xp @ Wk + bk)
  S  = q @ k.T + pos_bias    (diag forced very negative)
  topk16 per row of S/TEMP -> softmax weights w (zero elsewhere)
  v  = xp @ Wv + bv
  out[b] = (w / rowsum(w)) @ v                      # [P, D]
The multihead gather+combine with routes shared across heads is exactly a
row-sparse [P,P] x [P,D] matmul, so we compute it densely on the PE with a
masked-softmax weight matrix.

Sharding: data-parallel over batch B=8 across the 8 NeuronCores.

Precision: the top-16 boundary gaps reach ~1e-6, so the score path needs
~fp32 accuracy while fp16 matmul inputs only carry 11 mantissa bits. Each
score-path matmul therefore runs as a compensated pair:
  hi pass:    fp16 operands (1 cyc/row)
  cross pass: one fp8(e4m3) DoubleRow matmul computing lo.T@hi + hi.T@lo
              at 0.5 cyc/row, with lo pre-scaled by 2^13 to stay in e4m3
              range; the 2^-13 unscale is folded into the PSUM->SBUF copy.
This gives ~1e-7 normalized score error (empirically rel_err ~6e-3 end to
end) at half the PE cost of 3-pass compensated fp16. The x/W splits are
prepared on the host; the q/k splits are computed on-device from the
projection PSUM. V projection and the combine only contribute smooth
rounding error and run as plain fp16 single passes.

K is projected before Q so K's norm chain (gpsimd partition reduce + rsqrt
+ broadcast relayout) hides under Q's and V's PE time; the scores pipeline
runs tails two blocks behind the score matmuls to cover the ~11us
epilogue chain (PSUM combine -> rk/pb -> top-16 -> exp) latency.
"""

import numpy as np
import ml_dtypes

import concourse.mybir as mybir
import concourse.tile as tile
from concourse import bass_isa
from concourse import bacc
from concourse.bass_utils import run_bass_kernel_spmd
from concourse.masks import make_identity

F32 = mybir.dt.float32
F16 = mybir.dt.float16
F8 = mybir.dt.float8e4
AF = mybir.ActivationFunctionType
OP = mybir.AluOpType
DR = mybir.MatmulPerfMode.DoubleRow

B, P, D = 8, 1024, 768
TEMP = 0.1
KC = D // 128     # 6 contraction chunks
PB = P // 128     # 8 row blocks
MINVAL = -50.0    # match_replace fill; below any real score, above diag fill
DIAGVAL = -10000.0
S13 = float(2.0 ** 13)
S13I = float(2.0 ** -13)


def build_program(with_bias: bool):
    nc = bacc.Bacc(
        "TRN2",
        target_bir_lowering=False,
        debug=False,
        enable_asserts=False,
        num_devices=B,
    )
    xr_d = nc.dram_tensor("xr", [KC, 128, P], F16, kind="ExternalInput").ap()
    x8_d = nc.dram_tensor("x8", [KC, 128, 2, P], F8, kind="ExternalInput").ap()
    wqr_d = nc.dram_tensor("wqr", [KC, 128, D], F16, kind="ExternalInput").ap()
    wq8_d = nc.dram_tensor("wq8", [KC, 128, 2, D], F8, kind="ExternalInput").ap()
    wkr_d = nc.dram_tensor("wkr", [KC, 128, D], F16, kind="ExternalInput").ap()
    wk8_d = nc.dram_tensor("wk8", [KC, 128, 2, D], F8, kind="ExternalInput").ap()
    wvr_d = nc.dram_tensor("wvr", [KC, 128, D], F16, kind="ExternalInput").ap()
    pb = nc.dram_tensor("pb", [P, P], F32, kind="ExternalInput").ap()
    if with_bias:
        bqkv = nc.dram_tensor("bqkv", [1, 3, D], F32, kind="ExternalInput").ap()
    out = nc.dram_tensor("out", [P, D], F32, kind="ExternalOutput").ap()

    with tile.TileContext(nc) as tc:
        consts = tc.alloc_tile_pool(name="consts", bufs=1)
        persist = tc.alloc_tile_pool(name="persist", bufs=1)
        wv_pool = tc.alloc_tile_pool(name="wv_pool", bufs=1)
        wq_pool = tc.alloc_tile_pool(name="wq_pool", bufs=1)
        wk_pool = tc.alloc_tile_pool(name="wk_pool", bufs=1)
        inp_pool = tc.alloc_tile_pool(name="inp", bufs=1)
        work2 = tc.alloc_tile_pool(name="work2", bufs=2)
        psum2 = tc.alloc_tile_pool(name="psum2", bufs=1, space="PSUM")

        warm_ones = consts.tile([1, 64], F32, tag="warm_ones")
        nc.vector.memset(warm_ones, 1.0)
        # warm-up matmuls: keep the PE busy through the initial input-DMA
        # wait so the HAM clock gate is at full rate when real work starts
        warm_ps = psum2.tile([128, P], F32, tag="mm_ps", name="warm_ps",
                             bufs=3)
        for _ in range(52):
            nc.tensor.matmul(warm_ps[0:1, 0:64], warm_ones[:, 0:1],
                             warm_ones, start=True, stop=True)
        ones_row = consts.tile([1, 512], F32, tag="ones_row")
        nc.vector.memset(ones_row, 1.0)
        ident = consts.tile([128, 128], F16, tag="ident")
        make_identity(nc, ident)

        # ---- load inputs: two coarse DMAs per tensor (few DGE configs,
        # while the first half still lands early enough to start the PE) ----
        x_sb = inp_pool.tile([128, KC, P], F16, tag="x_sb", name="x_sb")
        x8_sb = inp_pool.tile([128, KC, 2, P], F8, tag="x8_sb", name="x8_sb")
        wq_sb = wq_pool.tile([128, KC, D], F16, tag="wq_sb", name="wq_sb")
        wq8_sb = wq_pool.tile([128, KC, 2, D], F8, tag="wq8_sb", name="wq8_sb")
        wk_sb = wk_pool.tile([128, KC, D], F16, tag="wk_sb", name="wk_sb")
        wk8_sb = wk_pool.tile([128, KC, 2, D], F8, tag="wk8_sb", name="wk8_sb")
        wv_sb = wv_pool.tile([128, KC, D], F16, tag="wv_sb", name="wv_sb")

        # transfers share one DMA pipe in practice, so order them by first
        # PE use. V runs first and consumes x position-block by position-
        # block, so the whole K/Q input stream hides under V+K+Q PE time.
        # Alternate SP/ACT rings so DGE config latency overlaps too.
        xr_s = xr_d.rearrange("o p f -> p o f")
        wvr_s = wvr_d.rearrange("o p f -> p o f")
        nc.sync.dma_start(x_sb[:, :, 0:128], xr_s[:, :, 0:128])
        nc.scalar.dma_start(wv_sb[:, :, 0:512], wvr_s[:, :, 0:512])
        nc.sync.dma_start(wv_sb[:, :, 512:], wvr_s[:, :, 512:])
        for pblk in range(1, PB):
            pbs = slice(pblk * 128, (pblk + 1) * 128)
            eng = nc.scalar if pblk % 2 else nc.sync
            eng.dma_start(x_sb[:, :, pbs], xr_s[:, :, pbs])
        nc.sync.dma_start(wk_sb, wkr_d.rearrange("o p f -> p o f"))
        nc.scalar.dma_start(wk8_sb, wk8_d.rearrange("o p t f -> p o t f"))
        nc.sync.dma_start(x8_sb, x8_d.rearrange("o p t f -> p o t f"))
        nc.scalar.dma_start(wq_sb, wqr_d.rearrange("o p f -> p o f"))
        nc.sync.dma_start(wq8_sb, wq8_d.rearrange("o p t f -> p o t f"))
        if with_bias:
            bias_sb = consts.tile([1, 3, D], F32, tag="bias_sb")
            nc.sync.dma_start(bias_sb, bqkv)

        # ---- v projection first (fp16 single pass, [p, d] layout): it only
        # needs x + Wv, so it starts ~4us in and its PE time covers the
        # K/Q weight and fp8-pair DMA stream ----
        v_sb = persist.tile([128, PB, D], F16, tag="v_sb")
        for pblk in range(PB):
            mm_ps = psum2.tile([128, P], F32, tag="mm_ps", name="mm_ps", bufs=3)
            for dc in range(KC):
                for sl, s in ((0, slice(0, 512)), (1, slice(512, D))):
                    nc.tensor.matmul(
                        mm_ps[:, s],
                        x_sb[:, dc, pblk * 128:(pblk + 1) * 128],
                        wv_sb[:, dc, s],
                        start=(dc == 0),
                        stop=(dc == KC - 1) and not with_bias,
                    )
            if with_bias:
                for sl, s in ((0, slice(0, 512)), (1, slice(512, D))):
                    nc.tensor.matmul(
                        mm_ps[:, s],
                        ones_row[:, :128],
                        bias_sb[:, 2, s],
                        start=False,
                        stop=True,
                    )
            nc.scalar.activation(v_sb[:, pblk, :], mm_ps[:, :D], AF.Identity,
                                 scale=float(2.0 ** -6))

        # ---- k/q projections: fp16 hi pass + fp8 DoubleRow cross pass ----
        # q8p slot order: [ql8s, q8]; k8p slot order: [k8, kl8s]  so the
        # scores DoubleRow pass computes ql.k + q.kl in one instruction.
        # K first: its norm chain hides under Q-proj + V-proj PE time.
        qk_tiles = {}
        rinv_rows = {}
        for nm, (wr, w8) in (("k", (wk_sb, wk8_sb)), ("q", (wq_sb, wq8_sb))):
            ti = 0 if nm == "q" else 1
            lo_slot = 0 if nm == "q" else 1
            hi_slot = 1 - lo_slot
            r_sb = persist.tile([128, KC, P], F16, tag=f"{nm}r", name=f"{nm}r")
            p8_sb = persist.tile([128, KC, 2, P], F8, tag=f"{nm}8", name=f"{nm}8")
            qk_tiles[nm] = (r_sb, p8_sb)
            sq_acc = work2.tile([128, P], F32, tag="sq_acc", bufs=1)

            for dblk in range(KC):
                dbs = slice(dblk * 128, (dblk + 1) * 128)
                mm_ps = psum2.tile([128, P], F32, tag="mm_ps", name="mm_ps",
                                   bufs=3)
                # hi pass (operands pre-scaled so the product sits at 2^13)
                # and fp8 cross pass accumulate into the SAME psum group
                for dc in range(KC):
                    for sl in range(2):
                        s = slice(sl * 512, (sl + 1) * 512)
                        nc.tensor.matmul(
                            mm_ps[:, s],
                            wr[:, dc, dbs],
                            x_sb[:, dc, s],
                            start=(dc == 0),
                            stop=False,
                        )
                for dc in range(KC):
                    for sl in range(2):
                        s = slice(sl * 512, (sl + 1) * 512)
                        nc.tensor.matmul(
                            mm_ps[:, s],
                            w8[:, dc, :, dbs],
                            x8_sb[:, dc, :, s],
                            start=False,
                            stop=(dc == KC - 1) and not with_bias,
                            perf_mode=DR,
                        )
                if with_bias:
                    for sl in range(2):
                        s = slice(sl * 512, (sl + 1) * 512)
                        nc.tensor.matmul(
                            mm_ps[:, s],
                            bias_sb[:, ti, dbs],
                            ones_row,
                            start=False,
                            stop=True,
                        )
                # psum holds 2^13 * q. Epilogue (everything reads the psum):
                #   qf    = 2^-13 * ps          (full fp32, for norms)
                #   r_sb  = fp16(ps)            (2^13-scaled hi operand)
                #   ql8   = e4m3(ps - r_sb)     (2^13-scaled lo operand)
                #   q8    = e4m3(2^-8 * ps)     (2^5-scaled fp8 hi operand)
                nc.scalar.activation(r_sb[:, dblk, :], mm_ps, AF.Identity)
                nc.vector.scalar_tensor_tensor(
                    p8_sb[:, dblk, lo_slot, :], mm_ps, 1.0, r_sb[:, dblk, :],
                    op0=OP.mult, op1=OP.subtract)
                nc.scalar.activation(p8_sb[:, dblk, hi_slot, :], mm_ps,
                                     AF.Identity, scale=float(2.0 ** -8))
                qf = work2.tile([128, P], F32, tag="qf", bufs=3)
                nc.scalar.activation(qf, mm_ps, AF.Identity, scale=S13I)
                if dblk == 0:
                    nc.vector.tensor_mul(sq_acc, qf, qf)
                else:
                    sq_sb = work2.tile([128, P], F32, tag="sq_sb")
                    nc.vector.tensor_mul(sq_sb, qf, qf)
                    nc.gpsimd.tensor_add(sq_acc, sq_acc, sq_sb)
            # norm2 = sum over partitions of sq_acc (gpsimd tree reduce),
            # broadcast to all partitions; row 0 feeds the rsqrt chain.
            allr = work2.tile([128, P], F32, tag="allr", bufs=1)
            nc.gpsimd.partition_all_reduce(allr, sq_acc, channels=128,
                                           reduce_op=bass_isa.ReduceOp.add)
            norm2_sb = allr[0:1, :]
            # rinv = 1/sqrt(norm2): ACT Sqrt + the accurate DVE reciprocal
            # (short serial chain; the old Ln/Exp + Newton cost ~7us of
            # [1,P]-row ops and two activation-table reloads)
            sqr_row = work2.tile([1, P], F32, tag="sqr_row", bufs=1)
            nc.scalar.sqrt(sqr_row, norm2_sb)
            rinv_row = work2.tile([1, P], F32, tag=f"rinv_{nm}",
                                  name=f"rinv_{nm}", bufs=1)
            nc.vector.reciprocal(rinv_row, sqr_row)
            rinv_rows[nm] = rinv_row

        # ---- rk broadcast relayout (rinv_k finished during Q proj, so this
        # does not stall the in-order PE queue) ----
        rk_bcast = persist.tile([128, P], F32, tag="rk_bcast")
        bc_ps = psum2.tile([128, P], F32, tag="mm_ps", name="bc_ps", bufs=3)
        for sl in range(2):
            s = slice(sl * 512, (sl + 1) * 512)
            nc.tensor.matmul(bc_ps[:, s], ones_row[:, :128],
                             rinv_rows["k"][:, s], start=True, stop=True)
        nc.scalar.activation(rk_bcast, bc_ps, AF.Identity)
        rq_cols = persist.tile([128, PB], F32, tag="rq_cols")
        rq18_cols = persist.tile([128, PB], F32, tag="rq18_cols")
        rq26_cols = persist.tile([128, PB], F32, tag="rq26_cols")

        psum2.release()
        work2.release()
        inp_pool.release()
        wk_pool.release()
        wq_pool.release()
        wv_pool.release()

        # ---- per row-block: scores, top-16 softmax, combine ----
        qr_sb, q8p_sb = qk_tiles["q"]
        kr_sb, k8p_sb = qk_tiles["k"]
        work3 = tc.alloc_tile_pool(name="work3", bufs=2)
        psum3 = tc.alloc_tile_pool(name="psum3", bufs=1, space="PSUM")

        def emit_scores(pblk, mid_hook=None):
            """S matmuls + per-half combine/scale/bias; returns s_sb.
            mid_hook (block 0 only) emits the rq relayout after the half-0
            matmuls: its PE work slots into the stream here while its
            outputs are still emitted before any epilogue consumer."""
            pbs = slice(pblk * 128, (pblk + 1) * 128)
            pb_sb = work3.tile([128, P], F32, tag="pb_sb", bufs=3)
            nc.sync.dma_start(pb_sb, pb[pbs, :])
            s_sb = work3.tile([128, P], F32, tag="s_sb", bufs=3)
            # matmuls for both halves first
            pss = []
            for sl in range(2):
                s = slice(sl * 512, (sl + 1) * 512)
                s_ps = psum3.tile([128, 512], F32, tag="s_ps", name="s_ps",
                                  bufs=2)
                c_ps = psum3.tile([128, 512], F32, tag="c_ps", name="c_ps",
                                  bufs=2)
                for dc in range(KC):
                    nc.tensor.matmul(
                        s_ps,
                        qr_sb[:, dc, pbs],
                        kr_sb[:, dc, s],
                        start=(dc == 0),
                        stop=(dc == KC - 1),
                    )
                for dc in range(KC):
                    nc.tensor.matmul(
                        c_ps,
                        q8p_sb[:, dc, :, pbs],
                        k8p_sb[:, dc, :, s],
                        start=(dc == 0),
                        stop=(dc == KC - 1),
                        perf_mode=DR,
                    )
                pss.append((s, s_ps, c_ps))
                if mid_hook is not None:
                    mid_hook()
                    mid_hook = None
                # epilogue for this half (overlaps the next half's matmuls)
                crs = work3.tile([128, 512], F32, tag="crs3")
                nc.scalar.activation(crs, c_ps, AF.Identity,
                                     scale=rq18_cols[:, pblk:pblk + 1])
                nc.vector.scalar_tensor_tensor(
                    s_sb[:, s], s_ps, rq26_cols[:, pblk:pblk + 1], crs,
                    op0=OP.mult, op1=OP.add)
                nc.gpsimd.tensor_mul(s_sb[:, s], s_sb[:, s], rk_bcast[:, s])
                nc.gpsimd.tensor_add(s_sb[:, s], s_sb[:, s],
                                     pb_sb[:, s])
            return s_sb

        def emit_topk(pblk, s_sb):
            """top-16 per row: two rounds of max8 + match_replace, then exp"""
            m8a = work3.tile([128, 8], F32, tag="m8a", bufs=4)
            nc.vector.max(m8a, s_sb)
            sz1 = work3.tile([128, P], F32, tag="sz1")
            nc.vector.match_replace(sz1, in_to_replace=m8a, in_values=s_sb,
                                    imm_value=MINVAL)
            m8b = work3.tile([128, 8], F32, tag="m8b")
            nc.vector.max(m8b, sz1)
            sz2 = work3.tile([128, P], F32, tag="sz2")
            nc.vector.match_replace(sz2, in_to_replace=m8b, in_values=sz1,
                                    imm_value=MINVAL)
            # two-exp trick: w = exp((s-m)/T) - exp((sz2-m)/T).
            # Off-top entries give f == u (identical fp16 rounding) -> 0;
            # top-16 entries have u ~ exp(-500) -> 0. No [128,P] subtract of
            # the scores needed, and the denominators cancel exactly too.
            ebias = work3.tile([128, 1], F32, tag="ebias", bufs=4)
            nc.vector.tensor_scalar_mul(ebias, m8a[:, 0:1], -1.0 / TEMP)
            f_sb = work3.tile([128, P], F16, tag="f_sb")
            denf = work3.tile([128, 1], F32, tag="denf", bufs=4)
            nc.scalar.activation(f_sb, s_sb, AF.Exp, bias=ebias,
                                 scale=1.0 / TEMP, accum_out=denf)
            u_sb = work3.tile([128, P], F16, tag="u_sb")
            denu = work3.tile([128, 1], F32, tag="denu", bufs=4)
            nc.scalar.activation(u_sb, sz2, AF.Exp, bias=ebias,
                                 scale=1.0 / TEMP, accum_out=denu)
            w_sb = work3.tile([128, P], F16, tag="w_sb", bufs=4)
            nc.vector.tensor_sub(w_sb, f_sb, u_sb)
            den = work3.tile([128, 1], F32, tag="den", bufs=4)
            nc.vector.tensor_sub(den, denf, denu)
            rden = work3.tile([128, 1], F32, tag="rden", bufs=4)
            nc.vector.reciprocal(rden, den)
            return w_sb, rden

        def emit_transpose(pblk, w_sb):
            """PE transposes + the ACT copy out of PSUM; the combine is
            emitted a full pipeline step later so the PE never waits on
            the wT round-trip."""
            tp_ps = psum3.tile([128, P], F16, tag="tp_ps", name="tp_ps", bufs=2)
            for qc in range(PB):
                nc.tensor.transpose(
                    tp_ps[:, qc * 128:(qc + 1) * 128],
                    w_sb[:, qc * 128:(qc + 1) * 128],
                    ident,
                )
            wT_sb = work3.tile([128, P], F16, tag="wT_sb", bufs=2)
            nc.scalar.activation(wT_sb, tp_ps, AF.Identity)
            return wT_sb

        def emit_combine(pblk, wT_sb, rden):
            pbs = slice(pblk * 128, (pblk + 1) * 128)
            o_ps = psum3.tile([128, D], F32, tag="o_ps", name="o_ps", bufs=1)
            for qc in range(PB):
                for sl, s in ((0, slice(0, 512)), (1, slice(512, D))):
                    nc.tensor.matmul(
                        o_ps[:, s],
                        wT_sb[:, qc * 128:(qc + 1) * 128],
                        v_sb[:, qc, s],
                        start=(qc == 0),
                        stop=(qc == PB - 1),
                    )
            out_sb = work3.tile([128, D], F32, tag="out_sb")
            nc.scalar.activation(out_sb, o_ps, AF.Identity, scale=rden)
            nc.sync.dma_start(out[pbs, :], out_sb)

        def emit_rq_relayout():
            """rinv_q row -> per-partition columns; emitted after scores(0)'s
            matmuls so the PE queue isn't blocked waiting for the q norm
            chain (only block 0's stt actually needs rq)."""
            rq_ps = psum3.tile([128, 512], F32, tag="c_ps", name="rq_ps",
                               bufs=2)
            # one accumulation group: per-column start flags would re-mark
            # the whole psum zero-region and wipe the previous columns
            for j in range(PB):
                nc.tensor.matmul(
                    rq_ps[:, j:j + 1],
                    rinv_rows["q"][:, j * 128:(j + 1) * 128],
                    ones_row[:, 0:1],
                    start=(j == 0),
                    stop=(j == PB - 1),
                    skip_group_check=True,
                )
            nc.scalar.activation(rq_cols, rq_ps[:, :PB], AF.Identity)
            nc.vector.tensor_scalar_mul(rq18_cols, rq_cols, float(2.0 ** -18))
            nc.vector.tensor_scalar_mul(rq26_cols, rq_cols, float(2.0 ** -26))

        # 3-stage software pipeline: scores(i) | tail(i-2) | topk(i-1).
        # The per-half stt of block i is emitted before topk(i-1) so the DVE
        # queue never head-of-line blocks the score-PSUM ring, and each
        # block's weights are ready two block-periods before its PE tail.
        sbuf_q = []   # (pblk, s_sb) awaiting topk
        w_q = []      # (pblk, w_sb, rden) awaiting transpose
        c_q = []      # (pblk, wT_sb, rden) awaiting combine
        def step(pblk=None):
            if pblk is not None:
                hook = emit_rq_relayout if pblk == 0 else None
                sbuf_q.append((pblk, emit_scores(pblk, mid_hook=hook)))
            if len(w_q) > 1:
                bi, w_sb, rden = w_q.pop(0)
                c_q.append((bi, emit_transpose(bi, w_sb), rden))
            if len(c_q) > 1:
                bi, wT_sb, rden = c_q.pop(0)
                emit_combine(bi, wT_sb, rden)
            if len(sbuf_q) > 1:
                bi, s_sb = sbuf_q.pop(0)
                w_q.append((bi, *emit_topk(bi, s_sb)))
        for pblk in range(PB):
            step(pblk)
        # drain: thresholds drop to zero so the queues empty
        while sbuf_q or w_q or c_q:
            if sbuf_q:
                bi, s_sb = sbuf_q.pop(0)
                w_q.append((bi, *emit_topk(bi, s_sb)))
            if w_q:
                bi, w_sb, rden = w_q.pop(0)
                c_q.append((bi, emit_transpose(bi, w_sb), rden))
            if c_q:
                bi, wT_sb, rden = c_q.pop(0)
                emit_combine(bi, wT_sb, rden)

        work3.release()
        psum3.release()
        persist.release()
        consts.release()

    nc.finalize()
    return nc


_PROG_CACHE = {}


def _h16(a):
    return np.asarray(a, np.float32).astype(np.float16)


def _e4(a):
    return np.ascontiguousarray(np.asarray(a, np.float32).astype(
        ml_dtypes.float8_e4m3))


def _chunks(a):
    """[D, N] -> [KC, 128, N]"""
    return np.ascontiguousarray(a.reshape(KC, 128, -1))


def _pair(hi8, lo8):
    """two [D, N] fp8 -> [KC, 128, 2, N]"""
    return np.ascontiguousarray(
        np.stack([hi8.reshape(KC, 128, -1), lo8.reshape(KC, 128, -1)],
                 axis=2))


def _split_w(W):
    """weights: fp16 hi chunks scaled 2^7, fp8 pair chunks
    [slot0 = e4m3(2^16 Wl), slot1 = e4m3(2^2 W)] so both cross products
    land at 2^13 and accumulate straight into the hi psum."""
    Wh = _h16(W)
    Wl = W - Wh.astype(np.float32)
    Wh7 = _h16(Wh.astype(np.float32) * 2.0 ** 7)
    return _chunks(Wh7), _pair(_e4(2.0 ** 16 * Wl), _e4(2.0 ** 2 * W))


def kernel(**inputs) -> np.ndarray:
    x = np.ascontiguousarray(np.asarray(inputs["x"], dtype=np.float32))
    Wq = np.asarray(inputs["Wq"], dtype=np.float32)
    Wk = np.asarray(inputs["Wk"], dtype=np.float32)
    Wv = np.asarray(inputs["Wv"], dtype=np.float32)
    bq = np.asarray(inputs["bq"], dtype=np.float32)
    bk = np.asarray(inputs["bk"], dtype=np.float32)
    bv = np.asarray(inputs["bv"], dtype=np.float32)
    pos_bias = np.asarray(inputs["pos_bias"], dtype=np.float32)

    with_bias = bool(np.any(bq) or np.any(bk) or np.any(bv))

    # Diagonal is excluded by the reference (set to -1e9 before top-k); any
    # value below every real score gives the identical top-16 and weights.
    pb_adj = np.ascontiguousarray(pos_bias.copy())
    np.fill_diagonal(pb_adj, DIAGVAL)

    if with_bias not in _PROG_CACHE:
        _PROG_CACHE[with_bias] = build_program(with_bias)
    nc = _PROG_CACHE[with_bias]

    wqr, wq8 = _split_w(Wq)
    wkr, wk8 = _split_w(Wk)
    wvr = _chunks(_h16(Wv))

    in_maps = []
    for b in range(B):
        xT = np.ascontiguousarray(x[b, 1:, :].T)
        xr = _h16(xT)
        xl = xT - xr.astype(np.float32)
        xr6 = _h16(xr.astype(np.float32) * 2.0 ** 6)
        m = {
            "xr": _chunks(xr6),
            # slot0 = e4m3(2^-3 x) pairs W-slot0 (2^16 Wl); slot1 =
            # e4m3(2^11 xl) pairs W-slot1 (2^2 W): both products 2^13
            "x8": _pair(_e4(2.0 ** -3 * xT), _e4(2.0 ** 11 * xl)),
            "wqr": wqr, "wq8": wq8, "wkr": wkr, "wk8": wk8, "wvr": wvr,
            "pb": pb_adj,
        }
        if with_bias:
            m["bqkv"] = np.ascontiguousarray(
                np.stack([bq * 2.0 ** 13, bk * 2.0 ** 13, bv * 2.0 ** 6])[None]
            ).astype(np.float32)
        in_maps.append(m)

    res = run_bass_kernel_spmd(nc, in_maps, core_ids=list(range(B)))
    return np.stack([res.results[b]["out"] for b in range(B)]).astype(np.float32)
